# revision 18
# baseline (speedup 1.0000x reference)
"""Trainium2 Bass kernel for the BallActor GNN (EdgeConv over fully-connected
per-sample graphs, batch 1024 x 21 objects).

Key algorithmic facts exploited:
  * knn_actor K=20 over NOBJ=21 with self masked => the "kNN graph" is simply
    ALL ordered pairs (i, j != i); top_k is unnecessary and max-aggregation is
    order independent.
  * EdgeConv first layer is linear in [x_i, x_j - x_i]:
        h(i,j) = x_i @ (A - B) + x_j @ B + bm1   (Wm1 = [[A],[B]])
    so per-node terms u_i = x_i@(A-B), v_j = x_j@B are precomputed and each
    edge costs only an elementwise add + tanh + the second 128x128 matmul.
  * The class embedding path collapses to a 3-row table lookup, folded into
    u/v via one-hot rows (host precomputes F3 = tanh(tanh(emb)@We + be) and
    G = F3 @ W_cls).
  * Edges are enumerated as 20 cyclic shifts d=1..20: j = (i+d) mod 21.  With
    v stored duplicated along the object axis ([S, 41]), every shift is a
    single strided access pattern; msg columns align with agg columns.

Engine balance (driven by real NTFF traces of the previous version):
  * Shifts are processed in PAIRS: one DVE tensor_tensor takes the max of the
    two shifts' PSUM matmul outputs into an SBUF f32 pair-buffer (one PSUM
    pass instead of two), and the all-SBUF running max agg=max(agg,pair) runs
    on the otherwise-idle Pool (gpsimd) engine (no PSUM port, full SBUF
    access).  This halves phase-B DVE time vs a per-shift PSUM running max.
  * tanh runs on ACT as ONE [128, 2*2688] instruction per pair.
  * Phase A uses multi-row stationaries (Ws1 as [4,128], Gu/Gv as [3,128]) so
    each 336-col chunk needs 1-2 column streams instead of 4-7 rank-1 ones.
  * The actor head output is computed transposed (nodes on partitions) via
    data-stationary matmuls into a [128, 84] PSUM tile, so final activations
    use all 128 partitions; ba2 is added by accumulating ones-row x ba2-row
    into the same PSUM bank.

Sharding: pure data parallel over the batch: 1024 samples -> 8 cores x 128.
Params are replicated; outputs are concatenated on host.
"""

import os
import numpy as np
import ml_dtypes

BS = 1024
NOBJ = 21
HID = 128
EMB = 64
NCORES = 8
S = BS // NCORES          # samples per core
N = S * NOBJ              # nodes per core (2688)
F32 = np.float32
BF16 = ml_dtypes.bfloat16

ACH = 16 * NOBJ           # phase A chunk: 16 samples = 336 cols
BCH = 448                 # phase B/C matmul chunk (1 PSUM bank-slot holds 512)

# weight-pack column layout (single [128, WCOLS] tensor, one DMA)
_OFF_WS2 = 0
_OFF_WUS = 128
_OFF_WVS = 256
_OFF_WM2 = 384
_OFF_WA1 = 512
_OFF_WA2 = 640            # 4 cols
_OFF_WS1 = 644            # rows 0-3: Ws1 [4,128]
_OFF_GU = _OFF_WS1 + 128  # rows 0-2: Gu [3,128]
_OFF_GV = _OFF_GU + 128   # rows 0-2: Gv [3,128]
_OFF_ONE = _OFF_GV + 128  # row 0: ones [1,128]
_OFF_BA2 = _OFF_ONE + 128  # row 0: ba2 tiled 21x [1,84]
WCOLS = _OFF_BA2 + 84

_cache = {}


def _build_nc(edge_dt_name: str):
    import concourse.bass as bass  # noqa: F401
    import concourse.bacc as bacc
    import concourse.tile as tile
    from concourse import mybir

    dt = mybir.dt
    edt = getattr(dt, edge_dt_name)
    AF = mybir.ActivationFunctionType
    OP = mybir.AluOpType

    nc = bacc.Bacc("TRN2")

    # ---------------- DRAM I/O ----------------
    d_state = nc.dram_tensor("state", [S, 63], dt.float32, kind="ExternalInput")
    d_tar = nc.dram_tensor("tar", [S, NOBJ * 2], dt.float32, kind="ExternalInput")
    d_wpack = nc.dram_tensor("wpack", [HID, WCOLS], edt, kind="ExternalInput")
    # bias rows: bs1, bs2, bm1, bm2, ba1
    d_bias = nc.dram_tensor("biases", [5, HID], dt.float32, kind="ExternalInput")
    # output transposed: [4(ch), 128(p), 21(b)]; node index = 128*b + p
    d_out = nc.dram_tensor("out", [4, HID, NOBJ], dt.float32,
                           kind="ExternalOutput")

    with tile.TileContext(nc) as tc, \
         tc.tile_pool(name="per", bufs=1) as per, \
         tc.tile_pool(name="edge", bufs=3) as edge:

        # ---- persistent tiles ----
        wpack = per.tile([HID, WCOLS], edt, tag="wpack")
        nc.sync.dma_start(out=wpack, in_=d_wpack[:])
        w_Ws2 = wpack[:, _OFF_WS2:_OFF_WS2 + HID]
        w_WuS = wpack[:, _OFF_WUS:_OFF_WUS + HID]
        w_WvS = wpack[:, _OFF_WVS:_OFF_WVS + HID]
        w_Wm2 = wpack[:, _OFF_WM2:_OFF_WM2 + HID]
        w_Wa1 = wpack[:, _OFF_WA1:_OFF_WA1 + HID]
        w_Wa2 = wpack[:, _OFF_WA2:_OFF_WA2 + 4]
        w_Ws1 = wpack[0:4, _OFF_WS1:_OFF_WS1 + HID]
        w_Gu = wpack[0:3, _OFF_GU:_OFF_GU + HID]
        w_Gv = wpack[0:3, _OFF_GV:_OFF_GV + HID]
        w_one = wpack[0:1, _OFF_ONE:_OFF_ONE + HID]
        w_ba2 = wpack[0:1, _OFF_BA2:_OFF_BA2 + 84]

        # per-partition bias columns [HID, 5]
        bcol = per.tile([HID, 5], dt.float32, tag="bcol")
        nc.sync.dma_start(out=bcol, in_=d_bias[:].rearrange("b h -> h b"))
        bs1 = bcol[:, 0:1]
        bs2 = bcol[:, 1:2]
        bm1 = bcol[:, 2:3]
        bm2 = bcol[:, 3:4]
        ba1 = bcol[:, 4:5]

        u_sb = per.tile([HID, S, NOBJ], edt, tag="u_sb")
        v_ext = per.tile([HID, S, 2 * NOBJ - 1], edt, tag="v_ext")
        agg = per.tile([HID, N], edt, tag="agg")

        nrep = int(os.environ.get("BALL_REPEAT", "1"))
        for _rep in range(nrep):
          # ---- phase A: inputs -> node features u, v ----
          with tc.tile_pool(name="phA", bufs=1) as phA, \
               tc.tile_pool(name="psA", bufs=2, space="PSUM") as psA:

            state_nat = phA.tile([S, 63], dt.float32, tag="state_nat")
            nc.sync.dma_start(out=state_nat, in_=d_state[:])
            tar_nat = phA.tile([S, NOBJ * 2], dt.float32, tag="tar_nat")
            nc.sync.dma_start(out=tar_nat, in_=d_tar[:])

            # one-hot of category in natural layout (exact in bf16), moved
            # to channel-major via SWDGE right away so oh_nat's lifetime
            # closes before later tiles are allocated
            oh_nat = per.tile([S, 3, NOBJ], edt, tag="oh_nat")
            cats_nat = state_nat[:].rearrange("s (i k) -> s k i", k=3)[:, 2, :]
            for c in range(3):
                nc.vector.tensor_scalar(
                    out=oh_nat[:, c, :], in0=cats_nat, scalar1=float(c),
                    scalar2=None, op0=OP.is_equal)
            oh3 = phA.tile([3, S, NOBJ], edt, tag="oh3")
            for c in range(3):
                nc.gpsimd.dma_start(out=oh3[c:c + 1], in_=oh_nat[:, c, :])

            # tanh(tar) in natural layout (cheap: 42 elems/partition)
            ttar_nat = phA.tile([S, NOBJ * 2], dt.float32, tag="ttar_nat")
            nc.scalar.activation(out=ttar_nat, in_=tar_nat, func=AF.Tanh)

            # Stage spatial channels into a channel-blocked [s, k, i] tile
            # so the partition-collapse DMAs move contiguous 21-elem runs.
            st3 = state_nat[:].rearrange("s (i k) -> s k i", k=3)
            tt2 = ttar_nat[:].rearrange("s (i c) -> s c i", c=2)
            comb = phA.tile([S, 4, NOBJ], edt, tag="comb")
            nc.vector.tensor_copy(comb[:, 0:2, :], st3[:, 0:2, :])
            nc.vector.tensor_copy(comb[:, 2:4, :], tt2)
            spat4 = phA.tile([4, S, NOBJ], edt, tag="spat4")
            for c in range(4):
                nc.sync.dma_start(out=spat4[c:c + 1], in_=comb[:, c, :])
            spat_c = spat4[:].rearrange("k s i -> k (s i)")
            oh_c = oh3[:].rearrange("k s i -> k (s i)")

            h1 = phA.tile([HID, N], edt, tag="h1")
            feat = phA.tile([HID, N], edt, tag="feat")

            # 8 chunks of 336 cols; megatile [128, 4, 512] = 4 chunks = half
            def achunks(half):
                for cc in range(4):
                    k = half * 4 + cc
                    yield cc, slice(k * ACH, (k + 1) * ACH)

            stages = []
            for kind in ("h1", "feat", "u", "v"):
                for half in (0, 1):
                    stages.append((kind, half))
            for kind, half in stages:
                p = psA.tile([HID, 4, 512], dt.float32, tag="psA")
                pview = p[:, :, 0:ACH]
                hs = slice(half * (N // 2), (half + 1) * (N // 2))
                if kind == "h1":
                    for cc, cs in achunks(half):
                        nc.tensor.matmul(p[:, cc, 0:ACH], w_Ws1,
                                         spat_c[:, cs], start=True, stop=True)
                    nc.scalar.activation(
                        out=h1[:, hs].rearrange("c (k e) -> c k e", k=4),
                        in_=pview, func=AF.Tanh, bias=bs1)
                elif kind == "feat":
                    for cc, cs in achunks(half):
                        nc.tensor.matmul(p[:, cc, 0:ACH], w_Ws2,
                                         h1[:, cs], start=True, stop=True)
                    nc.scalar.activation(
                        out=feat[:, hs].rearrange("c (k e) -> c k e", k=4),
                        in_=pview, func=AF.Tanh, bias=bs2)
                elif kind == "u":
                    for cc, cs in achunks(half):
                        nc.tensor.matmul(p[:, cc, 0:ACH], w_WuS,
                                         feat[:, cs], start=True, stop=False)
                        nc.tensor.matmul(p[:, cc, 0:ACH], w_Gu,
                                         oh_c[:, cs], start=False, stop=True)
                    nc.vector.tensor_copy(
                        u_sb[:].rearrange("c s i -> c (s i)")[:, hs]
                        .rearrange("c (k e) -> c k e", k=4), pview)
                else:  # v
                    for cc, cs in achunks(half):
                        nc.tensor.matmul(p[:, cc, 0:ACH], w_WvS,
                                         feat[:, cs], start=True, stop=False)
                        nc.tensor.matmul(p[:, cc, 0:ACH], w_Gv,
                                         oh_c[:, cs], start=False, stop=True)
                    # dst AP (chunk, sample-in-chunk, i) stays affine because
                    # chunks are sample-aligned (16 samples each)
                    nc.scalar.activation(
                        out=v_ext[:, half * 64:(half + 1) * 64, 0:NOBJ]
                        .rearrange("c (k s) i -> c k s i", k=4),
                        in_=pview.rearrange("c k (s i) -> c k s i", i=NOBJ),
                        func=AF.Copy)
            # duplicate v columns so every cyclic shift is one strided AP
            nc.vector.tensor_copy(v_ext[:, :, NOBJ:], v_ext[:, :, 0:NOBJ - 1])

          # ---- phase B: 420 edges/sample via 20 shifts ----
          # Thirds 0..EVT-1 of each shift's msg columns are evacuated from
          # PSUM by ACT *through* tanh(.+bm2) (max commutes with monotone
          # tanh), leaving DVE a cheap 2x bf16 max; remaining thirds use the
          # direct 1x running max(agg, psum) on DVE.  agg cols < EVT*896 are
          # therefore already in x-space (tanh applied), the rest raw.
          EVT = int(os.environ.get("BALL_EVAC_THIRDS", "1"))
          # shifts whose u+v add runs on the Pool engine (default 0: Pool
          # shares an SBUF port with DVE and measurably slows every DVE op)
          pool_adds = int(os.environ.get("BALL_POOL_ADDS", "0"))
          add_on_pool = set()
          acc = 0
          for d in range(1, 21):
              acc += pool_adds
              if acc >= 20:
                  acc -= 20
                  add_on_pool.add(d)

          with tc.tile_pool(name="psB", bufs=4, space="PSUM") as psB:
            t_of = {}

            def produce(k):
                # pair k covers shifts d0=2k+1, d1=2k+2
                h2 = edge.tile([HID, 2, N], edt, tag="h2")
                for di, d in enumerate((2 * k + 1, 2 * k + 2)):
                    eng = nc.gpsimd if d in add_on_pool else nc.vector
                    eng.tensor_tensor(
                        out=h2[:, di, :].rearrange("c (s i) -> c s i", i=NOBJ),
                        in0=u_sb, in1=v_ext[:, :, d:d + NOBJ], op=OP.add)
                t2 = edge.tile([HID, 2, N], edt, tag="t2")
                nc.scalar.activation(
                    out=t2[:].rearrange("c d n -> c (d n)"),
                    in_=h2[:].rearrange("c d n -> c (d n)"),
                    func=AF.Tanh, bias=bm1)
                t_of[k] = t2

            def consume_shift(t2, di, d):
                for j in range(3):
                    p = psB.tile([HID, 2, 512], dt.float32, tag="msg")
                    for m in range(2):
                        c0 = j * 2 * BCH + m * BCH
                        nc.tensor.matmul(
                            p[:, m, 0:BCH], w_Wm2, t2[:, di, c0:c0 + BCH],
                            start=True, stop=True)
                    cols = slice(j * 2 * BCH, (j + 1) * 2 * BCH)
                    aggv = agg[:, cols].rearrange("c (m e) -> c m e", m=2)
                    pv = p[:, :, 0:BCH]
                    if j < EVT:
                        ev = edge.tile([HID, 2, BCH], edt, tag="ev")
                        nc.scalar.activation(out=ev, in_=pv, func=AF.Tanh,
                                             bias=bm2)
                        if d == 1:
                            nc.vector.tensor_copy(aggv, ev)
                        else:
                            nc.vector.tensor_tensor(out=aggv, in0=aggv,
                                                    in1=ev, op=OP.max)
                    else:
                        if d == 1:
                            nc.vector.tensor_copy(aggv, pv)
                        else:
                            nc.vector.tensor_tensor(out=aggv, in0=aggv,
                                                    in1=pv, op=OP.max)

            def consume(k):
                t2 = t_of.pop(k)
                for di, d in enumerate((2 * k + 1, 2 * k + 2)):
                    consume_shift(t2, di, d)

            LOOKAHEAD = 2
            for k in range(LOOKAHEAD):
                produce(k)
            for k in range(10):
                if k + LOOKAHEAD < 10:
                    produce(k + LOOKAHEAD)
                consume(k)
            del t_of

            # ---- phase C: actor head (transposed output) ----
            # agg cols < EVT*896 are already tanh'd (x-space); rest need it
            x = edge.tile([HID, N], edt, tag="h2")
            ecols = EVT * 2 * BCH
            if ecols:
                nc.vector.tensor_copy(x[:, 0:ecols], agg[:, 0:ecols])
            if ecols < N:
                nc.scalar.activation(out=x[:, ecols:], in_=agg[:, ecols:],
                                     func=AF.Tanh, bias=bm2)
            a1 = edge.tile([HID, N], edt, tag="t2")
            for j in range(3):  # thirds of 896 = 2 x 448
                p = psB.tile([HID, 2, 512], dt.float32, tag="msg")
                for m in range(2):
                    c0 = j * 2 * BCH + m * BCH
                    nc.tensor.matmul(p[:, m, 0:BCH], w_Wa1,
                                     x[:, c0:c0 + BCH], start=True, stop=True)
                nc.scalar.activation(
                    out=a1[:, j * 2 * BCH:(j + 1) * 2 * BCH]
                    .rearrange("c (m e) -> c m e", m=2),
                    in_=p[:, :, 0:BCH], func=AF.Tanh, bias=ba1)

            # y[p, 4b+c] = sum_ch a1[ch, 128b+p] * Wa2[ch, c]  (+ ba2)
            # one PSUM accumulation group over all 84 cols: opened and closed
            # by two rank-1 ones x (ba2/2) matmuls around the 21 data matmuls
            py = psB.tile([HID, 2, 512], dt.float32, tag="msg")
            y = py[:, 0, 0:84]
            nc.tensor.matmul(y, w_one, w_ba2, start=True, stop=False)
            for b in range(NOBJ):
                nc.tensor.matmul(py[:, 0, 4 * b:4 * b + 4],
                                 a1[:, 128 * b:128 * (b + 1)], w_Wa2,
                                 start=False, stop=False)
            nc.tensor.matmul(y, w_one, w_ba2, start=False, stop=True)
            yt = per.tile([HID, 84], dt.float32, tag="yt")
            nc.scalar.activation(out=yt, in_=y, func=AF.Tanh)

            # O[p, c*21+b]: c in (mu_x, mu_y, std_x, std_y)
            O = per.tile([HID, 4 * NOBJ], dt.float32, tag="O")
            ytv = yt[:].rearrange("p (b c) -> p b c", c=4)
            Ov = O[:].rearrange("p (c b) -> p c b", c=4)
            nc.vector.tensor_scalar(
                out=Ov[:, 0:2, :].rearrange("p c b -> p c b"),
                in0=ytv[:, :, 0:2].rearrange("p b c -> p c b"),
                scalar1=0.3, scalar2=None, op0=OP.mult)
            # std = exp(3.5*t - 1.5)
            cneg = per.tile([HID, 1], dt.float32, tag="cneg")
            nc.vector.memset(cneg, -1.5)
            nc.scalar.activation(
                out=Ov[:, 2:4, :].rearrange("p c b -> p c b"),
                in_=ytv[:, :, 2:4].rearrange("p b c -> p c b"),
                func=AF.Exp, bias=cneg, scale=3.5)

            # out[c, 128b+p] <- O[p, c*21+b]
            # contiguous 21-elem runs per (c, p) descriptor
            for c in range(4):
                nc.gpsimd.dma_start(
                    out=d_out[c], in_=O[:, c * NOBJ:(c + 1) * NOBJ])

    nc.finalize()
    return nc


def _prep_params(inputs, edge_np):
    """Host-side pure parameter transforms (weights only, O(param size))."""
    f = lambda k: np.asarray(inputs[k], F32)
    Wm1 = f("Wm1")
    A, B = Wm1[:192], Wm1[192:]
    F3 = np.tanh(np.tanh(f("emb_table")) @ f("We") + f("be"))
    Gu = F3 @ (A[128:] - B[128:])
    Gv = F3 @ B[128:]

    wpack = np.zeros((HID, WCOLS), F32)
    wpack[:, _OFF_WS2:_OFF_WS2 + HID] = f("Ws2")
    wpack[:, _OFF_WUS:_OFF_WUS + HID] = A[:128] - B[:128]
    wpack[:, _OFF_WVS:_OFF_WVS + HID] = B[:128]
    wpack[:, _OFF_WM2:_OFF_WM2 + HID] = f("Wm2")
    wpack[:, _OFF_WA1:_OFF_WA1 + HID] = f("Wa1")
    wpack[:, _OFF_WA2:_OFF_WA2 + 4] = f("Wa2")
    wpack[0:4, _OFF_WS1:_OFF_WS1 + HID] = f("Ws1")
    wpack[0:3, _OFF_GU:_OFF_GU + HID] = Gu
    wpack[0:3, _OFF_GV:_OFF_GV + HID] = Gv
    wpack[0, _OFF_ONE:_OFF_ONE + HID] = 1.0
    # applied twice (group open + close), so half each time
    wpack[0, _OFF_BA2:_OFF_BA2 + 84] = 0.5 * np.tile(f("ba2"), NOBJ)

    biases = np.stack([f("bs1"), f("bs2"), f("bm1"), f("bm2"), f("ba1")])
    return dict(
        wpack=np.ascontiguousarray(wpack.astype(edge_np)),
        biases=np.ascontiguousarray(biases),
    )


def kernel(**inputs):
    from concourse.bass_utils import run_bass_kernel_spmd

    edge_dt_name = os.environ.get("BALL_EDGE_DT", "bfloat16")
    trace = os.environ.get("BALL_TRACE", "0") == "1"

    key = (edge_dt_name, os.environ.get("BALL_REPEAT", "1"))
    if key not in _cache:
        _cache[key] = _build_nc(edge_dt_name)
    nc = _cache[key]

    edge_np = {"bfloat16": BF16, "float32": F32}[edge_dt_name]
    params = _prep_params(inputs, edge_np)

    state = np.ascontiguousarray(np.asarray(inputs["state_inp"], F32))
    tar = np.asarray(inputs["tar_scores"], F32).reshape(BS, NOBJ * 2)

    in_maps = []
    for c in range(NCORES):
        m = dict(params)
        m["state"] = state[c * S:(c + 1) * S]
        m["tar"] = np.ascontiguousarray(tar[c * S:(c + 1) * S])
        in_maps.append(m)

    res = run_bass_kernel_spmd(nc, in_maps, core_ids=list(range(NCORES)),
                               trace=trace)
    kernel.last_results = res

    # out[c] is [4, 128(p), 21(b)] with node index n = 128*b + p = 21*s + i
    outs = [
        res.results[c]["out"].transpose(0, 2, 1).reshape(4, S, NOBJ)
        for c in range(NCORES)
    ]
    full = np.concatenate(outs, axis=1)          # [4, BS, NOBJ]
    mu = np.ascontiguousarray(
        full[0:2].transpose(1, 2, 0).reshape(BS, 2 * NOBJ))
    std = np.ascontiguousarray(
        full[2:4].transpose(1, 2, 0).reshape(BS, 2 * NOBJ))
    return mu, std


# revision 19
# speedup vs baseline: 1.0003x; 1.0003x over previous
"""Trainium2 Bass kernel for the BallActor GNN (EdgeConv over fully-connected
per-sample graphs, batch 1024 x 21 objects).

Key algorithmic facts exploited:
  * knn_actor K=20 over NOBJ=21 with self masked => the "kNN graph" is simply
    ALL ordered pairs (i, j != i); top_k is unnecessary and max-aggregation is
    order independent.
  * EdgeConv first layer is linear in [x_i, x_j - x_i]:
        h(i,j) = x_i @ (A - B) + x_j @ B + bm1   (Wm1 = [[A],[B]])
    so per-node terms u_i = x_i@(A-B), v_j = x_j@B are precomputed and each
    edge costs only an elementwise add + tanh + the second 128x128 matmul.
  * The class embedding path collapses to a 3-row table lookup, folded into
    u/v via one-hot rows (host precomputes F3 = tanh(tanh(emb)@We + be) and
    G = F3 @ W_cls).
  * Edges are enumerated as 20 cyclic shifts d=1..20: j = (i+d) mod 21.  With
    v stored duplicated along the object axis ([S, 41]), every shift is a
    single strided access pattern; msg columns align with agg columns.

Engine balance (driven by real NTFF traces of the previous version):
  * Shifts are processed in PAIRS: one DVE tensor_tensor takes the max of the
    two shifts' PSUM matmul outputs into an SBUF f32 pair-buffer (one PSUM
    pass instead of two), and the all-SBUF running max agg=max(agg,pair) runs
    on the otherwise-idle Pool (gpsimd) engine (no PSUM port, full SBUF
    access).  This halves phase-B DVE time vs a per-shift PSUM running max.
  * tanh runs on ACT as ONE [128, 2*2688] instruction per pair.
  * Phase A uses multi-row stationaries (Ws1 as [4,128], Gu/Gv as [3,128]) so
    each 336-col chunk needs 1-2 column streams instead of 4-7 rank-1 ones.
  * The actor head output is computed transposed (nodes on partitions) via
    data-stationary matmuls into a [128, 84] PSUM tile, so final activations
    use all 128 partitions; ba2 is added by accumulating ones-row x ba2-row
    into the same PSUM bank.

Sharding: pure data parallel over the batch: 1024 samples -> 8 cores x 128.
Params are replicated; outputs are concatenated on host.
"""

import os
import numpy as np
import ml_dtypes

BS = 1024
NOBJ = 21
HID = 128
EMB = 64
NCORES = 8
S = BS // NCORES          # samples per core
N = S * NOBJ              # nodes per core (2688)
F32 = np.float32
BF16 = ml_dtypes.bfloat16

ACH = 16 * NOBJ           # phase A chunk: 16 samples = 336 cols
BCH = 448                 # phase B/C matmul chunk (1 PSUM bank-slot holds 512)

# weight-pack column layout (single [128, WCOLS] tensor, one DMA)
_OFF_WS2 = 0
_OFF_WUS = 128
_OFF_WVS = 256
_OFF_WM2 = 384
_OFF_WA1 = 512
_OFF_WA2 = 640            # 4 cols
_OFF_WS1 = 644            # rows 0-3: Ws1 [4,128]
_OFF_GU = _OFF_WS1 + 128  # rows 0-2: Gu [3,128]
_OFF_GV = _OFF_GU + 128   # rows 0-2: Gv [3,128]
_OFF_ONE = _OFF_GV + 128  # row 0: ones [1,128]
_OFF_BA2 = _OFF_ONE + 128  # row 0: ba2 tiled 21x [1,84]
WCOLS = _OFF_BA2 + 84

_cache = {}


def _build_nc(edge_dt_name: str):
    import concourse.bass as bass  # noqa: F401
    import concourse.bacc as bacc
    import concourse.tile as tile
    from concourse import mybir

    dt = mybir.dt
    edt = getattr(dt, edge_dt_name)
    AF = mybir.ActivationFunctionType
    OP = mybir.AluOpType

    nc = bacc.Bacc("TRN2")

    # ---------------- DRAM I/O ----------------
    d_state = nc.dram_tensor("state", [S, 63], dt.float32, kind="ExternalInput")
    d_tar = nc.dram_tensor("tar", [S, NOBJ * 2], dt.float32, kind="ExternalInput")
    d_wpack = nc.dram_tensor("wpack", [HID, WCOLS], edt, kind="ExternalInput")
    # bias rows: bs1, bs2, bm1, bm2, ba1
    d_bias = nc.dram_tensor("biases", [5, HID], dt.float32, kind="ExternalInput")
    # output transposed: [4(ch), 128(p), 21(b)]; node index = 128*b + p
    d_out = nc.dram_tensor("out", [4, HID, NOBJ], dt.float32,
                           kind="ExternalOutput")

    with tile.TileContext(nc) as tc, \
         tc.tile_pool(name="per", bufs=1) as per, \
         tc.tile_pool(name="edge", bufs=int(os.environ.get("BALL_EDGE_BUFS", "4"))) as edge:

        # ---- persistent tiles ----
        wpack = per.tile([HID, WCOLS], edt, tag="wpack")
        nc.sync.dma_start(out=wpack, in_=d_wpack[:])
        w_Ws2 = wpack[:, _OFF_WS2:_OFF_WS2 + HID]
        w_WuS = wpack[:, _OFF_WUS:_OFF_WUS + HID]
        w_WvS = wpack[:, _OFF_WVS:_OFF_WVS + HID]
        w_Wm2 = wpack[:, _OFF_WM2:_OFF_WM2 + HID]
        w_Wa1 = wpack[:, _OFF_WA1:_OFF_WA1 + HID]
        w_Wa2 = wpack[:, _OFF_WA2:_OFF_WA2 + 4]
        w_Ws1 = wpack[0:4, _OFF_WS1:_OFF_WS1 + HID]
        w_Gu = wpack[0:3, _OFF_GU:_OFF_GU + HID]
        w_Gv = wpack[0:3, _OFF_GV:_OFF_GV + HID]
        w_one = wpack[0:1, _OFF_ONE:_OFF_ONE + HID]
        w_ba2 = wpack[0:1, _OFF_BA2:_OFF_BA2 + 84]

        # per-partition bias columns [HID, 5]
        bcol = per.tile([HID, 5], dt.float32, tag="bcol")
        nc.sync.dma_start(out=bcol, in_=d_bias[:].rearrange("b h -> h b"))
        bs1 = bcol[:, 0:1]
        bs2 = bcol[:, 1:2]
        bm1 = bcol[:, 2:3]
        bm2 = bcol[:, 3:4]
        ba1 = bcol[:, 4:5]

        u_sb = per.tile([HID, S, NOBJ], edt, tag="u_sb")
        v_ext = per.tile([HID, S, 2 * NOBJ - 1], edt, tag="v_ext")
        agg = per.tile([HID, N], edt, tag="agg")

        nrep = int(os.environ.get("BALL_REPEAT", "1"))
        for _rep in range(nrep):
          # ---- phase A: inputs -> node features u, v ----
          with tc.tile_pool(name="phA", bufs=1) as phA, \
               tc.tile_pool(name="psA", bufs=2, space="PSUM") as psA:

            state_nat = phA.tile([S, 63], dt.float32, tag="state_nat")
            nc.sync.dma_start(out=state_nat, in_=d_state[:])
            tar_nat = phA.tile([S, NOBJ * 2], dt.float32, tag="tar_nat")
            nc.sync.dma_start(out=tar_nat, in_=d_tar[:])

            # one-hot of category in natural layout (exact in bf16), moved
            # to channel-major via SWDGE right away so oh_nat's lifetime
            # closes before later tiles are allocated
            oh_nat = per.tile([S, 3, NOBJ], edt, tag="oh_nat")
            cats_nat = state_nat[:].rearrange("s (i k) -> s k i", k=3)[:, 2, :]
            for c in range(3):
                nc.vector.tensor_scalar(
                    out=oh_nat[:, c, :], in0=cats_nat, scalar1=float(c),
                    scalar2=None, op0=OP.is_equal)
            oh3 = phA.tile([3, S, NOBJ], edt, tag="oh3")
            for c in range(3):
                nc.gpsimd.dma_start(out=oh3[c:c + 1], in_=oh_nat[:, c, :])

            # tanh(tar) in natural layout (cheap: 42 elems/partition)
            ttar_nat = phA.tile([S, NOBJ * 2], dt.float32, tag="ttar_nat")
            nc.scalar.activation(out=ttar_nat, in_=tar_nat, func=AF.Tanh)

            # Stage spatial channels into a channel-blocked [s, k, i] tile
            # so the partition-collapse DMAs move contiguous 21-elem runs.
            st3 = state_nat[:].rearrange("s (i k) -> s k i", k=3)
            tt2 = ttar_nat[:].rearrange("s (i c) -> s c i", c=2)
            comb = phA.tile([S, 4, NOBJ], edt, tag="comb")
            nc.vector.tensor_copy(comb[:, 0:2, :], st3[:, 0:2, :])
            nc.vector.tensor_copy(comb[:, 2:4, :], tt2)
            spat4 = phA.tile([4, S, NOBJ], edt, tag="spat4")
            for c in range(4):
                nc.sync.dma_start(out=spat4[c:c + 1], in_=comb[:, c, :])
            spat_c = spat4[:].rearrange("k s i -> k (s i)")
            oh_c = oh3[:].rearrange("k s i -> k (s i)")

            h1 = phA.tile([HID, N], edt, tag="h1")
            feat = phA.tile([HID, N], edt, tag="feat")

            # 8 chunks of 336 cols; megatile [128, 4, 512] = 4 chunks = half
            def achunks(half):
                for cc in range(4):
                    k = half * 4 + cc
                    yield cc, slice(k * ACH, (k + 1) * ACH)

            stages = []
            for kind in ("h1", "feat", "u", "v"):
                for half in (0, 1):
                    stages.append((kind, half))
            for kind, half in stages:
                p = psA.tile([HID, 4, 512], dt.float32, tag="psA")
                pview = p[:, :, 0:ACH]
                hs = slice(half * (N // 2), (half + 1) * (N // 2))
                if kind == "h1":
                    for cc, cs in achunks(half):
                        nc.tensor.matmul(p[:, cc, 0:ACH], w_Ws1,
                                         spat_c[:, cs], start=True, stop=True)
                    nc.scalar.activation(
                        out=h1[:, hs].rearrange("c (k e) -> c k e", k=4),
                        in_=pview, func=AF.Tanh, bias=bs1)
                elif kind == "feat":
                    for cc, cs in achunks(half):
                        nc.tensor.matmul(p[:, cc, 0:ACH], w_Ws2,
                                         h1[:, cs], start=True, stop=True)
                    nc.scalar.activation(
                        out=feat[:, hs].rearrange("c (k e) -> c k e", k=4),
                        in_=pview, func=AF.Tanh, bias=bs2)
                elif kind == "u":
                    for cc, cs in achunks(half):
                        nc.tensor.matmul(p[:, cc, 0:ACH], w_WuS,
                                         feat[:, cs], start=True, stop=False)
                        nc.tensor.matmul(p[:, cc, 0:ACH], w_Gu,
                                         oh_c[:, cs], start=False, stop=True)
                    nc.vector.tensor_copy(
                        u_sb[:].rearrange("c s i -> c (s i)")[:, hs]
                        .rearrange("c (k e) -> c k e", k=4), pview)
                else:  # v
                    for cc, cs in achunks(half):
                        nc.tensor.matmul(p[:, cc, 0:ACH], w_WvS,
                                         feat[:, cs], start=True, stop=False)
                        nc.tensor.matmul(p[:, cc, 0:ACH], w_Gv,
                                         oh_c[:, cs], start=False, stop=True)
                    # dst AP (chunk, sample-in-chunk, i) stays affine because
                    # chunks are sample-aligned (16 samples each)
                    nc.scalar.activation(
                        out=v_ext[:, half * 64:(half + 1) * 64, 0:NOBJ]
                        .rearrange("c (k s) i -> c k s i", k=4),
                        in_=pview.rearrange("c k (s i) -> c k s i", i=NOBJ),
                        func=AF.Copy)
            # duplicate v columns so every cyclic shift is one strided AP
            nc.vector.tensor_copy(v_ext[:, :, NOBJ:], v_ext[:, :, 0:NOBJ - 1])

          # ---- phase B: 420 edges/sample via 20 shifts ----
          # Thirds 0..EVT-1 of each shift's msg columns are evacuated from
          # PSUM by ACT *through* tanh(.+bm2) (max commutes with monotone
          # tanh), leaving DVE a cheap 2x bf16 max; remaining thirds use the
          # direct 1x running max(agg, psum) on DVE.  agg cols < EVT*896 are
          # therefore already in x-space (tanh applied), the rest raw.
          EVT = int(os.environ.get("BALL_EVAC_THIRDS", "1"))
          # shifts whose u+v add runs on the Pool engine (default 0: Pool
          # shares an SBUF port with DVE and measurably slows every DVE op)
          pool_adds = int(os.environ.get("BALL_POOL_ADDS", "0"))
          add_on_pool = set()
          acc = 0
          for d in range(1, 21):
              acc += pool_adds
              if acc >= 20:
                  acc -= 20
                  add_on_pool.add(d)

          with tc.tile_pool(name="psB", bufs=4, space="PSUM") as psB:
            t_of = {}

            def produce(k):
                # pair k covers shifts d0=2k+1, d1=2k+2
                h2 = edge.tile([HID, 2, N], edt, tag="h2")
                for di, d in enumerate((2 * k + 1, 2 * k + 2)):
                    eng = nc.gpsimd if d in add_on_pool else nc.vector
                    eng.tensor_tensor(
                        out=h2[:, di, :].rearrange("c (s i) -> c s i", i=NOBJ),
                        in0=u_sb, in1=v_ext[:, :, d:d + NOBJ], op=OP.add)
                t2 = edge.tile([HID, 2, N], edt, tag="t2")
                nc.scalar.activation(
                    out=t2[:].rearrange("c d n -> c (d n)"),
                    in_=h2[:].rearrange("c d n -> c (d n)"),
                    func=AF.Tanh, bias=bm1)
                t_of[k] = t2

            def consume_shift(t2, di, d):
                for j in range(3):
                    p = psB.tile([HID, 2, 512], dt.float32, tag="msg")
                    for m in range(2):
                        c0 = j * 2 * BCH + m * BCH
                        nc.tensor.matmul(
                            p[:, m, 0:BCH], w_Wm2, t2[:, di, c0:c0 + BCH],
                            start=True, stop=True)
                    cols = slice(j * 2 * BCH, (j + 1) * 2 * BCH)
                    aggv = agg[:, cols].rearrange("c (m e) -> c m e", m=2)
                    pv = p[:, :, 0:BCH]
                    if j < EVT:
                        ev = edge.tile([HID, 2, BCH], edt, tag="ev")
                        nc.scalar.activation(out=ev, in_=pv, func=AF.Tanh,
                                             bias=bm2)
                        if d == 1:
                            nc.vector.tensor_copy(aggv, ev)
                        else:
                            nc.vector.tensor_tensor(out=aggv, in0=aggv,
                                                    in1=ev, op=OP.max)
                    else:
                        if d == 1:
                            nc.vector.tensor_copy(aggv, pv)
                        else:
                            nc.vector.tensor_tensor(out=aggv, in0=aggv,
                                                    in1=pv, op=OP.max)

            def consume(k):
                t2 = t_of.pop(k)
                for di, d in enumerate((2 * k + 1, 2 * k + 2)):
                    consume_shift(t2, di, d)

            LOOKAHEAD = int(os.environ.get("BALL_LOOKAHEAD", "3"))
            for k in range(LOOKAHEAD):
                produce(k)
            for k in range(10):
                if k + LOOKAHEAD < 10:
                    produce(k + LOOKAHEAD)
                consume(k)
            del t_of

            # ---- phase C: actor head (transposed output) ----
            # agg cols < EVT*896 are already tanh'd (x-space); rest need it
            x = edge.tile([HID, N], edt, tag="h2")
            ecols = EVT * 2 * BCH
            if ecols:
                nc.vector.tensor_copy(x[:, 0:ecols], agg[:, 0:ecols])
            if ecols < N:
                nc.scalar.activation(out=x[:, ecols:], in_=agg[:, ecols:],
                                     func=AF.Tanh, bias=bm2)
            a1 = edge.tile([HID, N], edt, tag="t2")
            for j in range(3):  # thirds of 896 = 2 x 448
                p = psB.tile([HID, 2, 512], dt.float32, tag="msg")
                for m in range(2):
                    c0 = j * 2 * BCH + m * BCH
                    nc.tensor.matmul(p[:, m, 0:BCH], w_Wa1,
                                     x[:, c0:c0 + BCH], start=True, stop=True)
                nc.scalar.activation(
                    out=a1[:, j * 2 * BCH:(j + 1) * 2 * BCH]
                    .rearrange("c (m e) -> c m e", m=2),
                    in_=p[:, :, 0:BCH], func=AF.Tanh, bias=ba1)

            # y[p, 4b+c] = sum_ch a1[ch, 128b+p] * Wa2[ch, c]  (+ ba2)
            # one PSUM accumulation group over all 84 cols: opened and closed
            # by two rank-1 ones x (ba2/2) matmuls around the 21 data matmuls
            py = psB.tile([HID, 2, 512], dt.float32, tag="msg")
            y = py[:, 0, 0:84]
            nc.tensor.matmul(y, w_one, w_ba2, start=True, stop=False)
            for b in range(NOBJ):
                nc.tensor.matmul(py[:, 0, 4 * b:4 * b + 4],
                                 a1[:, 128 * b:128 * (b + 1)], w_Wa2,
                                 start=False, stop=False)
            nc.tensor.matmul(y, w_one, w_ba2, start=False, stop=True)
            yt = per.tile([HID, 84], dt.float32, tag="yt")
            nc.scalar.activation(out=yt, in_=y, func=AF.Tanh)

            # O[p, c*21+b]: c in (mu_x, mu_y, std_x, std_y)
            O = per.tile([HID, 4 * NOBJ], dt.float32, tag="O")
            ytv = yt[:].rearrange("p (b c) -> p b c", c=4)
            Ov = O[:].rearrange("p (c b) -> p c b", c=4)
            nc.vector.tensor_scalar(
                out=Ov[:, 0:2, :].rearrange("p c b -> p c b"),
                in0=ytv[:, :, 0:2].rearrange("p b c -> p c b"),
                scalar1=0.3, scalar2=None, op0=OP.mult)
            # std = exp(3.5*t - 1.5)
            cneg = per.tile([HID, 1], dt.float32, tag="cneg")
            nc.vector.memset(cneg, -1.5)
            nc.scalar.activation(
                out=Ov[:, 2:4, :].rearrange("p c b -> p c b"),
                in_=ytv[:, :, 2:4].rearrange("p b c -> p c b"),
                func=AF.Exp, bias=cneg, scale=3.5)

            # out[c, 128b+p] <- O[p, c*21+b]
            # contiguous 21-elem runs per (c, p) descriptor
            for c in range(4):
                nc.gpsimd.dma_start(
                    out=d_out[c], in_=O[:, c * NOBJ:(c + 1) * NOBJ])

    nc.finalize()
    return nc


def _prep_params(inputs, edge_np):
    """Host-side pure parameter transforms (weights only, O(param size))."""
    f = lambda k: np.asarray(inputs[k], F32)
    Wm1 = f("Wm1")
    A, B = Wm1[:192], Wm1[192:]
    F3 = np.tanh(np.tanh(f("emb_table")) @ f("We") + f("be"))
    Gu = F3 @ (A[128:] - B[128:])
    Gv = F3 @ B[128:]

    wpack = np.zeros((HID, WCOLS), F32)
    wpack[:, _OFF_WS2:_OFF_WS2 + HID] = f("Ws2")
    wpack[:, _OFF_WUS:_OFF_WUS + HID] = A[:128] - B[:128]
    wpack[:, _OFF_WVS:_OFF_WVS + HID] = B[:128]
    wpack[:, _OFF_WM2:_OFF_WM2 + HID] = f("Wm2")
    wpack[:, _OFF_WA1:_OFF_WA1 + HID] = f("Wa1")
    wpack[:, _OFF_WA2:_OFF_WA2 + 4] = f("Wa2")
    wpack[0:4, _OFF_WS1:_OFF_WS1 + HID] = f("Ws1")
    wpack[0:3, _OFF_GU:_OFF_GU + HID] = Gu
    wpack[0:3, _OFF_GV:_OFF_GV + HID] = Gv
    wpack[0, _OFF_ONE:_OFF_ONE + HID] = 1.0
    # applied twice (group open + close), so half each time
    wpack[0, _OFF_BA2:_OFF_BA2 + 84] = 0.5 * np.tile(f("ba2"), NOBJ)

    biases = np.stack([f("bs1"), f("bs2"), f("bm1"), f("bm2"), f("ba1")])
    return dict(
        wpack=np.ascontiguousarray(wpack.astype(edge_np)),
        biases=np.ascontiguousarray(biases),
    )


def kernel(**inputs):
    from concourse.bass_utils import run_bass_kernel_spmd

    edge_dt_name = os.environ.get("BALL_EDGE_DT", "bfloat16")
    trace = os.environ.get("BALL_TRACE", "0") == "1"

    key = (edge_dt_name, os.environ.get("BALL_REPEAT", "1"))
    if key not in _cache:
        _cache[key] = _build_nc(edge_dt_name)
    nc = _cache[key]

    edge_np = {"bfloat16": BF16, "float32": F32}[edge_dt_name]
    params = _prep_params(inputs, edge_np)

    state = np.ascontiguousarray(np.asarray(inputs["state_inp"], F32))
    tar = np.asarray(inputs["tar_scores"], F32).reshape(BS, NOBJ * 2)

    in_maps = []
    for c in range(NCORES):
        m = dict(params)
        m["state"] = state[c * S:(c + 1) * S]
        m["tar"] = np.ascontiguousarray(tar[c * S:(c + 1) * S])
        in_maps.append(m)

    res = run_bass_kernel_spmd(nc, in_maps, core_ids=list(range(NCORES)),
                               trace=trace)
    kernel.last_results = res

    # out[c] is [4, 128(p), 21(b)] with node index n = 128*b + p = 21*s + i
    outs = [
        res.results[c]["out"].transpose(0, 2, 1).reshape(4, S, NOBJ)
        for c in range(NCORES)
    ]
    full = np.concatenate(outs, axis=1)          # [4, BS, NOBJ]
    mu = np.ascontiguousarray(
        full[0:2].transpose(1, 2, 0).reshape(BS, 2 * NOBJ))
    std = np.ascontiguousarray(
        full[2:4].transpose(1, 2, 0).reshape(BS, 2 * NOBJ))
    return mu, std


# revision 24
# speedup vs baseline: 1.0034x; 1.0031x over previous
"""Trainium2 Bass kernel for the BallActor GNN (EdgeConv over fully-connected
per-sample graphs, batch 1024 x 21 objects).

Key algorithmic facts exploited:
  * knn_actor K=20 over NOBJ=21 with self masked => the "kNN graph" is simply
    ALL ordered pairs (i, j != i); top_k is unnecessary and max-aggregation is
    order independent.
  * EdgeConv first layer is linear in [x_i, x_j - x_i]:
        h(i,j) = x_i @ (A - B) + x_j @ B + bm1   (Wm1 = [[A],[B]])
    so per-node terms u_i = x_i@(A-B), v_j = x_j@B are precomputed and each
    edge costs only an elementwise add + tanh + the second 128x128 matmul.
  * The class embedding path collapses to a 3-row table lookup, folded into
    u/v via one-hot rows (host precomputes F3 = tanh(tanh(emb)@We + be) and
    G = F3 @ W_cls).
  * Edges are enumerated as 20 cyclic shifts d=1..20: j = (i+d) mod 21.  With
    v stored duplicated along the object axis ([S, 41]), every shift is a
    single strided access pattern; msg columns align with agg columns.

Engine balance (driven by real NTFF traces of the previous version):
  * Shifts are processed in PAIRS: one DVE tensor_tensor takes the max of the
    two shifts' PSUM matmul outputs into an SBUF f32 pair-buffer (one PSUM
    pass instead of two), and the all-SBUF running max agg=max(agg,pair) runs
    on the otherwise-idle Pool (gpsimd) engine (no PSUM port, full SBUF
    access).  This halves phase-B DVE time vs a per-shift PSUM running max.
  * tanh runs on ACT as ONE [128, 2*2688] instruction per pair.
  * Phase A uses multi-row stationaries (Ws1 as [4,128], Gu/Gv as [3,128]) so
    each 336-col chunk needs 1-2 column streams instead of 4-7 rank-1 ones.
  * The actor head output is computed transposed (nodes on partitions) via
    data-stationary matmuls into a [128, 84] PSUM tile, so final activations
    use all 128 partitions; ba2 is added by accumulating ones-row x ba2-row
    into the same PSUM bank.

Sharding: pure data parallel over the batch: 1024 samples -> 8 cores x 128.
Params are replicated; outputs are concatenated on host.
"""

import os
import numpy as np
import ml_dtypes

BS = 1024
NOBJ = 21
HID = 128
EMB = 64
NCORES = 8
S = BS // NCORES          # samples per core
N = S * NOBJ              # nodes per core (2688)
F32 = np.float32
BF16 = ml_dtypes.bfloat16

ACH = 16 * NOBJ           # phase A chunk: 16 samples = 336 cols
BCH = 448                 # phase B/C matmul chunk (1 PSUM bank-slot holds 512)

# weight-pack column layout (single [128, WCOLS] tensor, one DMA)
_OFF_WS2 = 0
_OFF_WUS = 128
_OFF_WVS = 256
_OFF_WM2 = 384
_OFF_WA1 = 512
_OFF_WA2 = 640            # 4 cols
_OFF_WS1 = 644            # rows 0-3: Ws1 [4,128]
_OFF_GU = _OFF_WS1 + 128  # rows 0-2: Gu [3,128]
_OFF_GV = _OFF_GU + 128   # rows 0-2: Gv [3,128]
_OFF_ONE = _OFF_GV + 128  # row 0: ones [1,128]
_OFF_BA2 = _OFF_ONE + 128  # row 0: ba2 tiled 21x [1,84]
WCOLS = _OFF_BA2 + 84

_cache = {}


def _build_nc(edge_dt_name: str):
    import concourse.bass as bass  # noqa: F401
    import concourse.bacc as bacc
    import concourse.tile as tile
    from concourse import mybir

    dt = mybir.dt
    edt = getattr(dt, edge_dt_name)
    AF = mybir.ActivationFunctionType
    OP = mybir.AluOpType

    nc = bacc.Bacc("TRN2")

    # ---------------- DRAM I/O ----------------
    d_state = nc.dram_tensor("state", [S, 63], dt.float32, kind="ExternalInput")
    d_tar = nc.dram_tensor("tar", [S, NOBJ * 2], dt.float32, kind="ExternalInput")
    d_wpack = nc.dram_tensor("wpack", [HID, WCOLS], edt, kind="ExternalInput")
    # bias columns (pre-transposed on host): bs1, bs2, bm1, bm2, ba1
    d_bias = nc.dram_tensor("biases", [HID, 5], dt.float32, kind="ExternalInput")
    # output transposed: [4(ch), 128(p), 21(b)]; node index = 128*b + p
    d_out = nc.dram_tensor("out", [4, HID, NOBJ], dt.float32,
                           kind="ExternalOutput")

    with tile.TileContext(nc) as tc, \
         tc.tile_pool(name="per", bufs=1) as per, \
         tc.tile_pool(name="edge", bufs=int(os.environ.get("BALL_EDGE_BUFS", "4"))) as edge:

        # ---- persistent tiles ----
        # wpack is 360KB; keep it off the SP queue so the small input DMAs
        # (state/tar, issued first in phase A) are not stuck behind it
        wpack = per.tile([HID, WCOLS], edt, tag="wpack")
        nc.scalar.dma_start(out=wpack, in_=d_wpack[:])
        w_Ws2 = wpack[:, _OFF_WS2:_OFF_WS2 + HID]
        w_WuS = wpack[:, _OFF_WUS:_OFF_WUS + HID]
        w_WvS = wpack[:, _OFF_WVS:_OFF_WVS + HID]
        w_Wm2 = wpack[:, _OFF_WM2:_OFF_WM2 + HID]
        w_Wa1 = wpack[:, _OFF_WA1:_OFF_WA1 + HID]
        w_Wa2 = wpack[:, _OFF_WA2:_OFF_WA2 + 4]
        w_Ws1 = wpack[0:4, _OFF_WS1:_OFF_WS1 + HID]
        w_Gu = wpack[0:3, _OFF_GU:_OFF_GU + HID]
        w_Gv = wpack[0:3, _OFF_GV:_OFF_GV + HID]
        w_one = wpack[0:1, _OFF_ONE:_OFF_ONE + HID]
        w_ba2 = wpack[0:1, _OFF_BA2:_OFF_BA2 + 84]

        # per-partition bias columns [HID, 5]
        bcol = per.tile([HID, 5], dt.float32, tag="bcol")
        nc.sync.dma_start(out=bcol, in_=d_bias[:])
        bs1 = bcol[:, 0:1]
        bs2 = bcol[:, 1:2]
        bm1 = bcol[:, 2:3]
        bm2 = bcol[:, 3:4]
        ba1 = bcol[:, 4:5]

        u_sb = per.tile([HID, S, NOBJ], edt, tag="u_sb")
        v_ext = per.tile([HID, S, 2 * NOBJ - 1], edt, tag="v_ext")
        agg = per.tile([HID, N], edt, tag="agg")

        nrep = int(os.environ.get("BALL_REPEAT", "1"))
        for _rep in range(nrep):
          # ---- phase A: inputs -> node features u, v ----
          with tc.tile_pool(name="phA", bufs=1) as phA, \
               tc.tile_pool(name="psA", bufs=2, space="PSUM") as psA:

            state_nat = phA.tile([S, 63], dt.float32, tag="state_nat")
            nc.sync.dma_start(out=state_nat, in_=d_state[:])
            tar_nat = phA.tile([S, NOBJ * 2], dt.float32, tag="tar_nat")
            nc.sync.dma_start(out=tar_nat, in_=d_tar[:])

            # one-hot of category in natural layout (exact in bf16), moved
            # to channel-major via SWDGE right away so oh_nat's lifetime
            # closes before later tiles are allocated
            oh_nat = per.tile([S, 3, NOBJ], edt, tag="oh_nat")
            cats_nat = state_nat[:].rearrange("s (i k) -> s k i", k=3)[:, 2, :]
            for c in range(3):
                nc.vector.tensor_scalar(
                    out=oh_nat[:, c, :], in0=cats_nat, scalar1=float(c),
                    scalar2=None, op0=OP.is_equal)
            oh3 = phA.tile([3, S, NOBJ], edt, tag="oh3")
            for c in range(3):
                nc.gpsimd.dma_start(out=oh3[c:c + 1], in_=oh_nat[:, c, :])

            # tanh(tar) in natural layout (cheap: 42 elems/partition)
            ttar_nat = phA.tile([S, NOBJ * 2], dt.float32, tag="ttar_nat")
            nc.scalar.activation(out=ttar_nat, in_=tar_nat, func=AF.Tanh)

            # Stage spatial channels into a channel-blocked [s, k, i] tile
            # so the partition-collapse DMAs move contiguous 21-elem runs.
            st3 = state_nat[:].rearrange("s (i k) -> s k i", k=3)
            tt2 = ttar_nat[:].rearrange("s (i c) -> s c i", c=2)
            comb = phA.tile([S, 4, NOBJ], edt, tag="comb")
            nc.vector.tensor_copy(comb[:, 0:2, :], st3[:, 0:2, :])
            nc.vector.tensor_copy(comb[:, 2:4, :], tt2)
            spat4 = phA.tile([4, S, NOBJ], edt, tag="spat4")
            for c in range(4):
                nc.sync.dma_start(out=spat4[c:c + 1], in_=comb[:, c, :])
            spat_c = spat4[:].rearrange("k s i -> k (s i)")
            oh_c = oh3[:].rearrange("k s i -> k (s i)")

            h1 = phA.tile([HID, N], edt, tag="h1")
            feat = phA.tile([HID, N], edt, tag="feat")

            # 8 chunks of 336 cols; megatile [128, 4, 512] = 4 chunks = half
            def achunks(half):
                for cc in range(4):
                    k = half * 4 + cc
                    yield cc, slice(k * ACH, (k + 1) * ACH)

            stages = []
            for kind in ("h1", "feat", "u", "v"):
                for half in (0, 1):
                    stages.append((kind, half))
            for kind, half in stages:
                p = psA.tile([HID, 4, 512], dt.float32, tag="psA")
                pview = p[:, :, 0:ACH]
                hs = slice(half * (N // 2), (half + 1) * (N // 2))
                if kind == "h1":
                    for cc, cs in achunks(half):
                        nc.tensor.matmul(p[:, cc, 0:ACH], w_Ws1,
                                         spat_c[:, cs], start=True, stop=True)
                    nc.scalar.activation(
                        out=h1[:, hs].rearrange("c (k e) -> c k e", k=4),
                        in_=pview, func=AF.Tanh, bias=bs1)
                elif kind == "feat":
                    for cc, cs in achunks(half):
                        nc.tensor.matmul(p[:, cc, 0:ACH], w_Ws2,
                                         h1[:, cs], start=True, stop=True)
                    nc.scalar.activation(
                        out=feat[:, hs].rearrange("c (k e) -> c k e", k=4),
                        in_=pview, func=AF.Tanh, bias=bs2)
                elif kind == "u":
                    for cc, cs in achunks(half):
                        nc.tensor.matmul(p[:, cc, 0:ACH], w_WuS,
                                         feat[:, cs], start=True, stop=False)
                        nc.tensor.matmul(p[:, cc, 0:ACH], w_Gu,
                                         oh_c[:, cs], start=False, stop=True)
                    nc.vector.tensor_copy(
                        u_sb[:].rearrange("c s i -> c (s i)")[:, hs]
                        .rearrange("c (k e) -> c k e", k=4), pview)
                else:  # v
                    for cc, cs in achunks(half):
                        nc.tensor.matmul(p[:, cc, 0:ACH], w_WvS,
                                         feat[:, cs], start=True, stop=False)
                        nc.tensor.matmul(p[:, cc, 0:ACH], w_Gv,
                                         oh_c[:, cs], start=False, stop=True)
                    # dst AP (chunk, sample-in-chunk, i) stays affine because
                    # chunks are sample-aligned (16 samples each)
                    nc.scalar.activation(
                        out=v_ext[:, half * 64:(half + 1) * 64, 0:NOBJ]
                        .rearrange("c (k s) i -> c k s i", k=4),
                        in_=pview.rearrange("c k (s i) -> c k s i", i=NOBJ),
                        func=AF.Copy)
            # duplicate v columns so every cyclic shift is one strided AP
            nc.vector.tensor_copy(v_ext[:, :, NOBJ:], v_ext[:, :, 0:NOBJ - 1])

          # ---- phase B: 420 edges/sample via 20 shifts ----
          # Thirds 0..EVT-1 of each shift's msg columns are evacuated from
          # PSUM by ACT *through* tanh(.+bm2) (max commutes with monotone
          # tanh), leaving DVE a cheap 2x bf16 max; remaining thirds use the
          # direct 1x running max(agg, psum) on DVE.  agg cols < EVT*896 are
          # therefore already in x-space (tanh applied), the rest raw.
          EVT = int(os.environ.get("BALL_EVAC_THIRDS", "1"))
          # shifts whose u+v add runs on the Pool engine (default 0: Pool
          # shares an SBUF port with DVE and measurably slows every DVE op)
          pool_adds = int(os.environ.get("BALL_POOL_ADDS", "0"))
          add_on_pool = set()
          acc = 0
          for d in range(1, 21):
              acc += pool_adds
              if acc >= 20:
                  acc -= 20
                  add_on_pool.add(d)

          with tc.tile_pool(name="psB", bufs=4, space="PSUM") as psB:
            t_of = {}

            def produce(k):
                # pair k covers shifts d0=2k+1, d1=2k+2
                h2 = edge.tile([HID, 2, N], edt, tag="h2")
                for di, d in enumerate((2 * k + 1, 2 * k + 2)):
                    eng = nc.gpsimd if d in add_on_pool else nc.vector
                    eng.tensor_tensor(
                        out=h2[:, di, :].rearrange("c (s i) -> c s i", i=NOBJ),
                        in0=u_sb, in1=v_ext[:, :, d:d + NOBJ], op=OP.add)
                t2 = edge.tile([HID, 2, N], edt, tag="t2")
                nc.scalar.activation(
                    out=t2[:].rearrange("c d n -> c (d n)"),
                    in_=h2[:].rearrange("c d n -> c (d n)"),
                    func=AF.Tanh, bias=bm1)
                t_of[k] = t2

            def consume_shift(t2, di, d):
                for j in range(3):
                    p = psB.tile([HID, 2, 512], dt.float32, tag="msg")
                    for m in range(2):
                        c0 = j * 2 * BCH + m * BCH
                        nc.tensor.matmul(
                            p[:, m, 0:BCH], w_Wm2, t2[:, di, c0:c0 + BCH],
                            start=True, stop=True)
                    cols = slice(j * 2 * BCH, (j + 1) * 2 * BCH)
                    aggv = agg[:, cols].rearrange("c (m e) -> c m e", m=2)
                    pv = p[:, :, 0:BCH]
                    if j < EVT:
                        ev = edge.tile([HID, 2, BCH], edt, tag="ev")
                        nc.scalar.activation(out=ev, in_=pv, func=AF.Tanh,
                                             bias=bm2)
                        if d == 1:
                            nc.vector.tensor_copy(aggv, ev)
                        else:
                            nc.vector.tensor_tensor(out=aggv, in0=aggv,
                                                    in1=ev, op=OP.max)
                    else:
                        if d == 1:
                            nc.vector.tensor_copy(aggv, pv)
                        else:
                            nc.vector.tensor_tensor(out=aggv, in0=aggv,
                                                    in1=pv, op=OP.max)

            def consume(k):
                t2 = t_of.pop(k)
                for di, d in enumerate((2 * k + 1, 2 * k + 2)):
                    consume_shift(t2, di, d)

            LOOKAHEAD = int(os.environ.get("BALL_LOOKAHEAD", "3"))
            for k in range(LOOKAHEAD):
                produce(k)
            for k in range(10):
                if k + LOOKAHEAD < 10:
                    produce(k + LOOKAHEAD)
                consume(k)
            del t_of

            # ---- phase C: actor head (transposed output) ----
            # agg cols < EVT*896 are already tanh'd (x-space); rest need it
            x = edge.tile([HID, N], edt, tag="h2")
            ecols = EVT * 2 * BCH
            if ecols:
                nc.vector.tensor_copy(x[:, 0:ecols], agg[:, 0:ecols])
            if ecols < N:
                nc.scalar.activation(out=x[:, ecols:], in_=agg[:, ecols:],
                                     func=AF.Tanh, bias=bm2)
            a1 = edge.tile([HID, N], edt, tag="t2")
            for j in range(3):  # thirds of 896 = 2 x 448
                p = psB.tile([HID, 2, 512], dt.float32, tag="msg")
                for m in range(2):
                    c0 = j * 2 * BCH + m * BCH
                    nc.tensor.matmul(p[:, m, 0:BCH], w_Wa1,
                                     x[:, c0:c0 + BCH], start=True, stop=True)
                nc.scalar.activation(
                    out=a1[:, j * 2 * BCH:(j + 1) * 2 * BCH]
                    .rearrange("c (m e) -> c m e", m=2),
                    in_=p[:, :, 0:BCH], func=AF.Tanh, bias=ba1)

            # y[p, 4b+c] = sum_ch a1[ch, 128b+p] * Wa2[ch, c]  (+ ba2)
            # one PSUM accumulation group over all 84 cols: opened and closed
            # by two rank-1 ones x (ba2/2) matmuls around the 21 data matmuls
            py = psB.tile([HID, 2, 512], dt.float32, tag="msg")
            y = py[:, 0, 0:84]
            nc.tensor.matmul(y, w_one, w_ba2, start=True, stop=False)
            for b in range(NOBJ):
                nc.tensor.matmul(py[:, 0, 4 * b:4 * b + 4],
                                 a1[:, 128 * b:128 * (b + 1)], w_Wa2,
                                 start=False, stop=False)
            nc.tensor.matmul(y, w_one, w_ba2, start=False, stop=True)
            yt = per.tile([HID, 84], dt.float32, tag="yt")
            nc.scalar.activation(out=yt, in_=y, func=AF.Tanh)

            # O[p, c*21+b]: c in (mu_x, mu_y, std_x, std_y)
            O = per.tile([HID, 4 * NOBJ], dt.float32, tag="O")
            ytv = yt[:].rearrange("p (b c) -> p b c", c=4)
            Ov = O[:].rearrange("p (c b) -> p c b", c=4)
            nc.vector.tensor_scalar(
                out=Ov[:, 0:2, :].rearrange("p c b -> p c b"),
                in0=ytv[:, :, 0:2].rearrange("p b c -> p c b"),
                scalar1=0.3, scalar2=None, op0=OP.mult)
            # std = exp(3.5*t - 1.5)
            cneg = per.tile([HID, 1], dt.float32, tag="cneg")
            nc.vector.memset(cneg, -1.5)
            nc.scalar.activation(
                out=Ov[:, 2:4, :].rearrange("p c b -> p c b"),
                in_=ytv[:, :, 2:4].rearrange("p b c -> p c b"),
                func=AF.Exp, bias=cneg, scale=3.5)

            # out[c, 128b+p] <- O[p, c*21+b]
            # contiguous 21-elem runs per (c, p) descriptor
            for c in range(4):
                nc.gpsimd.dma_start(
                    out=d_out[c], in_=O[:, c * NOBJ:(c + 1) * NOBJ])

    nc.finalize()
    return nc


def _prep_params(inputs, edge_np):
    """Host-side pure parameter transforms (weights only, O(param size))."""
    f = lambda k: np.asarray(inputs[k], F32)
    Wm1 = f("Wm1")
    A, B = Wm1[:192], Wm1[192:]
    F3 = np.tanh(np.tanh(f("emb_table")) @ f("We") + f("be"))
    Gu = F3 @ (A[128:] - B[128:])
    Gv = F3 @ B[128:]

    wpack = np.zeros((HID, WCOLS), F32)
    wpack[:, _OFF_WS2:_OFF_WS2 + HID] = f("Ws2")
    wpack[:, _OFF_WUS:_OFF_WUS + HID] = A[:128] - B[:128]
    wpack[:, _OFF_WVS:_OFF_WVS + HID] = B[:128]
    wpack[:, _OFF_WM2:_OFF_WM2 + HID] = f("Wm2")
    wpack[:, _OFF_WA1:_OFF_WA1 + HID] = f("Wa1")
    wpack[:, _OFF_WA2:_OFF_WA2 + 4] = f("Wa2")
    wpack[0:4, _OFF_WS1:_OFF_WS1 + HID] = f("Ws1")
    wpack[0:3, _OFF_GU:_OFF_GU + HID] = Gu
    wpack[0:3, _OFF_GV:_OFF_GV + HID] = Gv
    wpack[0, _OFF_ONE:_OFF_ONE + HID] = 1.0
    # applied twice (group open + close), so half each time
    wpack[0, _OFF_BA2:_OFF_BA2 + 84] = 0.5 * np.tile(f("ba2"), NOBJ)

    biases = np.stack([f("bs1"), f("bs2"), f("bm1"), f("bm2"), f("ba1")])
    return dict(
        wpack=np.ascontiguousarray(wpack.astype(edge_np)),
        biases=np.ascontiguousarray(biases.T),
    )


def kernel(**inputs):
    from concourse.bass_utils import run_bass_kernel_spmd

    edge_dt_name = os.environ.get("BALL_EDGE_DT", "bfloat16")
    trace = os.environ.get("BALL_TRACE", "0") == "1"

    key = (edge_dt_name, os.environ.get("BALL_REPEAT", "1"))
    if key not in _cache:
        _cache[key] = _build_nc(edge_dt_name)
    nc = _cache[key]

    edge_np = {"bfloat16": BF16, "float32": F32}[edge_dt_name]
    params = _prep_params(inputs, edge_np)

    state = np.ascontiguousarray(np.asarray(inputs["state_inp"], F32))
    tar = np.asarray(inputs["tar_scores"], F32).reshape(BS, NOBJ * 2)

    in_maps = []
    for c in range(NCORES):
        m = dict(params)
        m["state"] = state[c * S:(c + 1) * S]
        m["tar"] = np.ascontiguousarray(tar[c * S:(c + 1) * S])
        in_maps.append(m)

    res = run_bass_kernel_spmd(nc, in_maps, core_ids=list(range(NCORES)),
                               trace=trace)
    kernel.last_results = res

    # out[c] is [4, 128(p), 21(b)] with node index n = 128*b + p = 21*s + i
    outs = [
        res.results[c]["out"].transpose(0, 2, 1).reshape(4, S, NOBJ)
        for c in range(NCORES)
    ]
    full = np.concatenate(outs, axis=1)          # [4, BS, NOBJ]
    mu = np.ascontiguousarray(
        full[0:2].transpose(1, 2, 0).reshape(BS, 2 * NOBJ))
    std = np.ascontiguousarray(
        full[2:4].transpose(1, 2, 0).reshape(BS, 2 * NOBJ))
    return mu, std
